# revision 46
# baseline (speedup 1.0000x reference)
"""Trainium2 Bass kernel for nn_AssociativeMemory (Hopfield recall).

Computes state <- tanh(W @ state) for 10 iterations, W: [8192, 8192] f32.

Strategy (8 NeuronCores, SPMD):
  - Row-shard W: core r owns rows [r*1024, (r+1)*1024).
  - fp16 hi/lo split of W and state (host-side for W): ~22 effective
    mantissa bits at 1 col/cycle PE rate (true fp32 matmul is 4x
    slower).  Final rel err ~1e-3 -- the intrinsic fp32 noise floor of
    this chaotic recurrence.
  - Scale design: Wl = fp16((W - Wh) * 2^6); A-pass stationary
    [sh, sl*2^12], B-pass stationary [sh*2^-6, sl*2^6].  A0/B0 share
    scale 1 and A1/B1 share scale 2^12, so all matmuls accumulate into
    ONE [2, 1024] PSUM tile; y = row0 + row1/2^12.  Every scale is a
    power of two and keeps all fp16 values normal (no subnormal flush).
  - Pipelined halves: each iteration computes output half 0 then half
    1 (128 matmuls each).  Half-0's pre-activations AllGather while
    the PE computes half 1; the next iteration starts on the k-chunks
    half-0's gather delivered (k-halves map to chunk halves via
    k = r*1024 + h*512 + q*32 + c', chunk c = h*32 + c', partition
    p = r*16 + q), and reaches the half-1-dependent chunks after that
    gather lands.  Steady state: the PE never idles; collectives,
    tanh and hi/lo splits all hide under matmuls.
  - State stationaries are double-buffered across iterations (no WAR
    between in-flight splits and current-iteration reads).
  - 64 + RESIDENT_WL W chunk-units stay SBUF-resident across all 10
    iterations; the rest of Wl streams from HBM each half in
    [128, 2chunk, 512] pieces.
"""

import numpy as np

import concourse.mybir as mybir
import concourse.tile as tile
from concourse import bacc
from concourse.bass_utils import run_bass_kernel_spmd

P = 8192
N_CORES = 8
ROWS = P // N_CORES          # 1024 output rows per core
NPART = 128                  # SBUF partitions / PE contraction size
CHUNKS = P // NPART          # 64 contraction chunks
HCHUNKS = CHUNKS // 2        # chunks per k-half
HALF = 512                   # output half width / PE moving free-dim
ITERATIONS = 10
SL_SCALE = 4096.0            # 2^12
WL_SCALE = 64.0              # 2^6
EPS = 1.0 / SL_SCALE

RESIDENT_WL = 26             # Wl chunks resident in SBUF (of 64)

# Iterations that run the full hi+lo W product.  Later iterations use only
# Wh: error injected at iteration t is amplified ~2.2x per remaining
# iteration, so the final iterations tolerate fp16-hi-only precision
# (2 hi-only iters: 1.6e-3 rel, 1.6e-2 absmax -- equal to the fp32
# self-noise envelope of this recurrence).
FULL_ITERS = 8

_CACHED = {}


def _build_nc():
    # Bacc (not raw Bass): its generate_event_semaphores pass splits
    # multi-wait instructions (HW allows 1 wait/inst) via event semaphores.
    nc = bacc.Bacc(None, target_bir_lowering=False)
    f16 = mybir.dt.float16
    f32 = mybir.dt.float32

    xin = nc.dram_tensor("xin", [P], f32, kind="ExternalInput")
    wh = nc.dram_tensor("wh", [NPART, CHUNKS, ROWS], f16, kind="ExternalInput")
    wl = nc.dram_tensor("wl", [NPART, CHUNKS, ROWS], f16, kind="ExternalInput")
    # each core writes only its own row-slice; the host concatenates
    out = nc.dram_tensor("out", [ROWS], f32, kind="ExternalOutput")
    # [1, eps] column for the final-iteration PSUM row combine on the PE
    cvec = nc.inline_tensor(np.array([[1.0], [EPS]], dtype=np.float32), name="cvec")

    with tile.TileContext(nc) as tc:
        with (
            tc.tile_pool(name="wres", bufs=1) as wres,
            tc.tile_pool(name="stream", bufs=8) as stream,
            tc.tile_pool(name="state", bufs=1) as state,
            tc.tile_pool(name="tmp", bufs=2) as tmp,
            tc.tile_pool(name="psum", bufs=2, space="PSUM") as psum,
            tc.tile_pool(name="dram", bufs=1, space="DRAM") as dram,
        ):
            # resident weights; loads issued just-in-time inside iteration 0
            wh_sb = wres.tile([NPART, CHUNKS, ROWS], f16)
            wl_sb = wres.tile([NPART, RESIDENT_WL, ROWS], f16)

            # state stationaries, double-buffered across iterations:
            # s_a[b] = [sh, sl*2^12], s_b[b] = [sh*2^-6, sl*2^6]
            s_a = [state.tile([NPART, 2, CHUNKS], f16, name=f"s_a{b}") for b in (0, 1)]
            s_b = [state.tile([NPART, 2, CHUNKS], f16, name=f"s_b{b}") for b in (0, 1)]

            def split_state(src_f32, buf, csl):
                """hi/lo split of [128, n] state into chunk-slice csl of buf."""
                d_full = tmp.tile([NPART, CHUNKS], f32, tag="d", name="d_full")
                d = d_full[:, csl]
                sa, sb = s_a[buf], s_b[buf]
                nc.vector.tensor_copy(sa[:, 0, csl], src_f32[:])
                nc.vector.tensor_tensor(
                    d, src_f32[:], sa[:, 0, csl], mybir.AluOpType.subtract
                )
                nc.vector.tensor_scalar_mul(sa[:, 1, csl], d, SL_SCALE)
                nc.vector.tensor_scalar_mul(sb[:, 0, csl], sa[:, 0, csl], 1.0 / WL_SCALE)
                nc.vector.tensor_scalar_mul(sb[:, 1, csl], d, WL_SCALE)

            # initial split of x into buffer 0 (no tanh on iteration-1 input)
            x_sb = state.tile([NPART, CHUNKS], f32)
            nc.sync.dma_start(x_sb[:], xin.rearrange("(p c) -> p c", p=NPART))
            split_state(x_sb, 0, slice(0, CHUNKS))

            cvec_sb = state.tile([2, 1], f32)
            nc.sync.dma_start(cvec_sb[:], cvec[:])

            def gather_tail(it, h, acc):
                """AllGather output-half h of iteration `it`, then tanh and
                split into state buffer (it+1)%2 chunk-half h (hidden under
                subsequent matmuls).  The final iteration skips the gather:
                combine rows on the PE, tanh, and write this core's slice."""
                osl = slice(h * HALF, (h + 1) * HALF)
                u_sb = tmp.tile([2, HALF], f32, tag="u_sb")
                nc.scalar.activation(
                    u_sb[:], acc[:, osl], mybir.ActivationFunctionType.Copy
                )
                if it == ITERATIONS - 1:
                    yf = psum.tile([1, HALF], f32, tag="yf")
                    nc.tensor.matmul(yf[:], cvec_sb[:], u_sb[:], start=True, stop=True)
                    yt = tmp.tile([1, HALF], f32, tag="yt")
                    nc.scalar.activation(
                        yt[:], yf[:], mybir.ActivationFunctionType.Tanh
                    )
                    nc.sync.dma_start(out.rearrange("(a b) -> a b", a=1)[:, osl], yt[:])
                    return
                cc_in = dram.tile([2, HALF], f32, name=f"cc_in_{it}_{h}")
                cc_out = dram.tile(
                    [N_CORES, 2, HALF], f32, addr_space="Shared",
                    name=f"cc_out_{it}_{h}",
                )
                nc.sync.dma_start(cc_in[:], u_sb[:])
                nc.gpsimd.collective_compute(
                    "AllGather",
                    mybir.AluOpType.bypass,
                    replica_groups=[list(range(N_CORES))],
                    ins=[cc_in[:]],
                    outs=[cc_out[:]],
                )
                # reload at [128, 2, 32]; 8 per-rank DMAs over both HWDGE
                # engines (k = r*1024 + h*512 + q*32 + c')
                u2 = tmp.tile([NPART, 2, HCHUNKS], f32, tag="u2")
                q = NPART // N_CORES
                for r in range(N_CORES):
                    eng = nc.sync if r % 2 == 0 else nc.scalar
                    eng.dma_start(
                        u2[r * q : (r + 1) * q, :, :],
                        cc_out[r, :, :].rearrange("j (q c) -> q j c", c=HCHUNKS),
                    )
                s_pre = tmp.tile([NPART, HCHUNKS], f32, tag="s_pre")
                nc.vector.scalar_tensor_tensor(
                    s_pre[:],
                    u2[:, 1, :],
                    EPS,
                    u2[:, 0, :],
                    mybir.AluOpType.mult,
                    mybir.AluOpType.add,
                )
                s_f = tmp.tile([NPART, HCHUNKS], f32, tag="s_f")
                nc.scalar.activation(
                    s_f[:], s_pre[:], mybir.ActivationFunctionType.Tanh
                )
                csl = slice(h * HCHUNKS, (h + 1) * HCHUNKS)
                split_state(s_f, (it + 1) % 2, csl)

            for it in range(ITERATIONS):
                buf = it % 2
                acc = psum.tile([2, ROWS], f32, tag="acc")
                for h in range(2):
                    osl = slice(h * HALF, (h + 1) * HALF)
                    # natural chunk order: chunks < HCHUNKS depend on the
                    # half-0 gather (landed), the rest on the half-1 gather
                    # (in flight at the start of each h0) -- the PE reaches
                    # them ~14us later, after it lands.
                    corder = list(range(CHUNKS))
                    full = it < FULL_ITERS
                    for c in corder:
                        if it == 0:
                            eng = nc.sync if c % 2 == 0 else nc.gpsimd
                            eng.dma_start(wh_sb[:, c, osl], wh[:, c, osl])
                            if c < RESIDENT_WL:
                                eng.dma_start(wl_sb[:, c, osl], wl[:, c, osl])
                        if not full:
                            wl_rhs = None
                        elif c < RESIDENT_WL:
                            wl_rhs = wl_sb[:, c, osl]
                        else:
                            ci = c - RESIDENT_WL
                            if ci % 2 == 0:
                                wl_t = stream.tile(
                                    [NPART, 2, HALF], f16, tag="wl_t"
                                )
                                nc.gpsimd.dma_start(wl_t[:], wl[:, c : c + 2, osl])
                                wl_rhs = wl_t[:, 0, :]
                            else:
                                wl_rhs = wl_t[:, 1, :]
                        first = c == corder[0]
                        last = c == corder[-1]
                        nc.tensor.matmul(
                            acc[:, osl],
                            s_a[buf][:, :, c],
                            wh_sb[:, c, osl],
                            start=first,
                            stop=(last and not full),
                        )
                        if full:
                            nc.tensor.matmul(
                                acc[:, osl],
                                s_b[buf][:, :, c],
                                wl_rhs,
                                start=False,
                                stop=last,
                            )
                    gather_tail(it, h, acc)
    nc.compile()
    return nc


def _prepare_in_maps(x, weights):
    x = np.ascontiguousarray(x, dtype=np.float32)
    w32 = np.asarray(weights, dtype=np.float32)
    # k-map: k = r*1024 + h*512 + q*32 + c'  <->  p = r*16+q, c = h*32+c'
    in_maps = []
    for r in range(N_CORES):
        wt = np.ascontiguousarray(w32[r * ROWS : (r + 1) * ROWS, :].T)  # [8192, 1024]
        whi = wt.astype(np.float16)
        wlo = ((wt - whi.astype(np.float32)) * WL_SCALE).astype(np.float16)

        def remap(a):
            # [8192 k, 1024 i] -> [128 p, 64 c, 1024 i]
            a = a.reshape(N_CORES, 2, NPART // N_CORES, HCHUNKS, ROWS)
            a = a.transpose(0, 2, 1, 3, 4)  # r, q, h, c', i
            return np.ascontiguousarray(a.reshape(NPART, CHUNKS, ROWS))

        in_maps.append({"xin": _permute_x(x), "wh": remap(whi), "wl": remap(wlo)})
    return in_maps


def _permute_x(x):
    # xin DMA loads [p, c] as x[p*64 + c]; give it x in the k-map order:
    # position p*64+c must hold x[k(p, c)]
    k = _kmap()
    return np.ascontiguousarray(x[k].reshape(-1))


def _kmap():
    p = np.arange(NPART)[:, None]
    c = np.arange(CHUNKS)[None, :]
    r, q = p // 16, p % 16
    h, cp = c // HCHUNKS, c % HCHUNKS
    return (r * ROWS + h * HALF + q * HCHUNKS + cp).reshape(NPART, CHUNKS)


def _run(inputs, **kwargs):
    if "nc" not in _CACHED:
        _CACHED["nc"] = _build_nc()
    nc = _CACHED["nc"]
    in_maps = _prepare_in_maps(inputs["x"], inputs["weights"])
    res = run_bass_kernel_spmd(nc, in_maps, core_ids=list(range(N_CORES)), **kwargs)
    out = np.concatenate([np.asarray(res.results[r]["out"]) for r in range(N_CORES)])
    return np.ascontiguousarray(out, dtype=np.float32), res


def kernel(**inputs) -> np.ndarray:
    out, _ = _run(inputs)
    return out
